# revision 25
# baseline (speedup 1.0000x reference)
"""Trainium2 Bass kernel for nn_BackpropKalmanFilter.

Math: after the Riccati recursion converges (t >= T1, ~320 steps), the Kalman
gain is constant and the filter is the LTI recursion
    x_t = A x_{t-1} + K z_t,   A = (I - K H) F  (contractive)
||A^d|| < 6e-5 for d >= D (=256), so each output is a finite convolution over
the last D measurements.  Blocked form with block L=32, J = D/L = 8:
    U_k = C Z_k                        (block response, C: 16 x 256)
    Y_k = G Z_k + sum_{j=1..8} (Pm A32^{j-1}) U_{k-j}
All per-block, fully parallel; T is sharded over 8 cores with a J-block halo;
the pre-convergence transient (t < T0) is computed exactly on the host and
overwrites the first rows.

Device notes: PE column rate is ~0.83ns/col for k>=64 and ~0.42ns/col for
k=16, so minimize column instances: G at k=128 with block-triangular skip
(6 matmuls/stripe).  The U-history tile ust[16q+t,c] = U[t,c+q] is built with
8 small SBUF->SBUF DMAs; KAL_XFOLD=1 collapses it to block-end states with
one k=128 matmul so the per-row-tile application runs at k=16.  All matmul
operands stay at SBUF base partition 0 (changing base partition inside a PSUM
accumulation group hangs the PE).
"""
import os
import sys

import numpy as np

sys.path.insert(0, "/opt/trn_rl_repo")
sys.path.insert(0, "/root/.axon_site")
sys.path.insert(0, "/root/.axon_site/_ro/pypackages")

N, M = 16, 8          # state / measurement dims
T = 500_000
L = 32                # block length
J = 8                 # halo blocks (D = J*L = 256 decay length)
D = J * L
NCORES = 8
KB = 1960                         # blocks per core
TTOT = NCORES * KB * L            # 501760 padded steps
KC = KB + J                       # 1968 columns incl. halo
SWS = [160, 500, 500, 500, 300]   # stripe widths (small first/last for
                                  # early start and short drain tail)
DTYPE_MODE = os.environ.get("KAL_DTYPE", "bf16")   # bf16 | fp32
OUT_MODE = os.environ.get("KAL_OUT", "bf16")       # bf16 | fp32
XFOLD = int(os.environ.get("KAL_XFOLD", "0"))      # 0: wu k=128, 1: fold+pm
REPS = int(os.environ.get("KAL_REPS", "1"))        # timing amplification
HWLOOP = int(os.environ.get("KAL_HWLOOP", "0"))    # hardware-loop reps

# packed weight tensor [128, WTS_COLS]: wg0 | wg1 | wu | wx | ct0 | ct1 | pm
WG1_OFF = 512
WU_OFF = 1024
WX_OFF = 1536
CT_OFF = 1552
PM_OFF = 1584
A32_OFF = 2096        # a32T in rows 0:16
WTS_COLS = 2224

_cache = {}


# ----------------------------------------------------------------- host math
def _riccati(F, H, Q, R):
    F64, H64 = F.astype(np.float64), H.astype(np.float64)
    Q64, R64 = Q.astype(np.float64), R.astype(np.float64)
    P = np.eye(N)
    prevK = None
    T1 = None
    for t in range(1024):
        P = F64 @ P @ F64.T + Q64
        S = H64 @ P @ H64.T + R64
        K = P @ H64.T @ np.linalg.inv(S)
        P = (np.eye(N) - K @ H64) @ P
        if prevK is not None and T1 is None and np.abs(K - prevK).max() < 1e-13:
            T1 = t
        prevK = K
    assert T1 is not None
    A = (np.eye(N) - K @ H64) @ F64
    return K, A, T1


def _build_weights(F, H, Q, R):
    K_ss, A, T1 = _riccati(F, H, Q, R)
    npow = L * J + 2
    Apow = np.empty((npow, N, N))
    Apow[0] = np.eye(N)
    for i in range(1, npow):
        Apow[i] = Apow[i - 1] @ A
    AK = Apow @ K_ss                                   # A^d K

    C = np.concatenate([AK[L - 1 - j] for j in range(L)], axis=1)  # (16, 256)

    G = np.zeros((N * L, M * L))
    for i in range(L):
        for j in range(i + 1):
            G[i * N:(i + 1) * N, j * M:(j + 1) * M] = AK[i - j]
    Pm = np.concatenate([Apow[i + 1] for i in range(L)], axis=0)   # (512, 16)
    # ust group q holds U_{k-(8-q)}  ->  PB col group q = Pm A^{32(7-q)}
    PB = np.concatenate([Pm @ Apow[L * (7 - q)] for q in range(J)],
                        axis=1)                                    # (512, 128)
    # X-fold: X_{k-1} = sum_q A32^{7-q} ust_group_q
    WX = np.concatenate([Apow[L * (7 - q)].T for q in range(J)], axis=0)
    # (128, 16): rows 16q+p hold A32^{7-q}[.,p]

    wts = np.zeros((128, WTS_COLS), np.float32)
    wts[:, 0:512] = G[:, 0:128].T.reshape(128, 512)
    wts[:, WG1_OFF:WG1_OFF + 512] = G[:, 128:256].T.reshape(128, 512)
    wts[:, WU_OFF:WU_OFF + 512] = PB.T
    wts[:, WX_OFF:WX_OFF + 16] = WX
    wts[:, CT_OFF:CT_OFF + 16] = C[:, :128].T
    wts[:, CT_OFF + 16:CT_OFF + 32] = C[:, 128:].T
    wts[:16, PM_OFF:PM_OFF + 512] = Pm.T
    wts[:16, A32_OFF:A32_OFF + 128] = np.concatenate(
        [Apow[L * jj].T for jj in range(J)], axis=1)

    T0 = ((T1 + D) + L - 1) // L * L
    return wts, T0


def _host_transient(meas, F, H, Q, R, T0):
    F64, H64 = F.astype(np.float64), H.astype(np.float64)
    Q64, R64 = Q.astype(np.float64), R.astype(np.float64)
    x = np.zeros(N)
    P = np.eye(N)
    out = np.empty((T0, N))
    for t in range(T0):
        x = F64 @ x
        P = F64 @ P @ F64.T + Q64
        z = meas[t, :, 0].astype(np.float64)
        S = H64 @ P @ H64.T + R64
        K = P @ H64.T @ np.linalg.inv(S)
        x = x + K @ (z - H64 @ x)
        P = (np.eye(N) - K @ H64) @ P
        out[t] = x
    return out


def _chunks(total):
    assert sum(SWS) == total
    out = []
    s = 0
    for w in SWS:
        out.append((s, w))
        s += w
    return out


# ------------------------------------------------------------- device program
def _build_program(dtype_mode, out_mode, xfold):
    import concourse.bacc as bacc
    import concourse.bass as bass
    import concourse.tile as tile
    from concourse import mybir

    f32 = mybir.dt.float32
    cdt = mybir.dt.bfloat16 if dtype_mode == "bf16" else f32
    odt = mybir.dt.bfloat16 if out_mode == "bf16" else f32

    nc = bacc.Bacc("TRN2", target_bir_lowering=False, debug=False,
                   enable_asserts=False, num_devices=NCORES)

    zmat_d = nc.dram_tensor("zmat", [128, 2 * KC], cdt,
                            kind="ExternalInput").ap()
    wts_d = nc.dram_tensor("wts", [128, WTS_COLS], cdt,
                           kind="ExternalInput").ap()
    out_d = nc.dram_tensor("out", [4, 128, KB], odt, kind="ExternalOutput").ap()

    stripes = _chunks(KB)
    nst = len(stripes)
    w0 = SWS[0] + 8
    with tile.TileContext(nc, trace_sim=False) as tc:
        with (
            tc.tile_pool(name="const", bufs=1) as const,
            tc.tile_pool(name="up", bufs=3) as up,
            tc.tile_pool(name="vp", bufs=3) as vp,
            tc.tile_pool(name="usp", bufs=3) as usp,
            tc.tile_pool(name="ysp", bufs=8) as ysp,
            tc.tile_pool(name="psA", bufs=(1 if xfold else 2),
                         space=bass.MemorySpace.PSUM) as psA,
            tc.tile_pool(name="psV", bufs=(2 if xfold else 1),
                         space=bass.MemorySpace.PSUM) as psV,
            tc.tile_pool(name="psC", bufs=(5 if xfold else 6),
                         space=bass.MemorySpace.PSUM) as psC,
        ):
            zm = const.tile([128, 2 * KC], cdt, name="zm")
            wts = const.tile([128, WTS_COLS], cdt, name="wts")
            # weights in consumption order: ct (pass A), wg0+wu (stripe-0 C),
            # wg1, then the k=16 tails
            nc.scalar.dma_start(wts[:, CT_OFF:CT_OFF + 32],
                                wts_d[:, CT_OFF:CT_OFF + 32])
            nc.scalar.dma_start(wts[:, 0:512], wts_d[:, 0:512])
            nc.scalar.dma_start(wts[:, WU_OFF:WU_OFF + 512],
                                wts_d[:, WU_OFF:WU_OFF + 512])
            nc.scalar.dma_start(wts[:, WG1_OFF:WG1_OFF + 512],
                                wts_d[:, WG1_OFF:WG1_OFF + 512])
            nc.scalar.dma_start(wts[:, WX_OFF:CT_OFF],
                                wts_d[:, WX_OFF:CT_OFF])
            nc.scalar.dma_start(wts[:, PM_OFF:WTS_COLS],
                                wts_d[:, PM_OFF:WTS_COLS])
            # measurements: per-stripe chunks, ktile0 on sync, ktile1 on gpsimd
            bnds = [0]
            acc = 0
            for wsi in SWS:
                acc += wsi
                bnds.append(min(acc + 8, KC))
            for c0, c1 in zip(bnds[:-1], bnds[1:]):
                nc.sync.dma_start(zm[:, c0:c1], zmat_d[:, c0:c1])
                nc.gpsimd.dma_start(zm[:, KC + c0:KC + c1],
                                    zmat_d[:, KC + c0:KC + c1])

            ct = [wts[:, CT_OFF + 16 * i:CT_OFF + 16 * (i + 1)]
                  for i in range(2)]

            import contextlib
            loop_cm = tc.For_i(0, HWLOOP, 1) if HWLOOP else contextlib.nullcontext()
            with loop_cm:
                for rep in range(REPS):
                    for si, (s, w) in enumerate(stripes):
                        last = si == nst - 1
                        wj = w + J
                        # Pass A: U_k = C Z_k for blocks s-8 .. s+w-1
                        pu = psA.tile([16, 512], f32, name="pu")
                        nc.tensor.matmul(pu[:, :wj], ct[0], zm[:, s:s + wj],
                                         start=True, stop=False)
                        nc.tensor.matmul(pu[:, :wj], ct[1],
                                         zm[:, KC + s:KC + s + wj],
                                         start=False, stop=True)
                        u16 = up.tile([16, 512], cdt, name="u16")
                        nc.vector.tensor_copy(u16[:, :wj], pu[:, :wj])
                        if xfold != 2:
                            # shifted replica: ust[16q+t, c] = u16[t, c+q]
                            ust = usp.tile([128, 512], cdt, name="ust")
                            shift_engs = [nc.sync, nc.scalar,
                                          nc.gpsimd, nc.sync]
                            for q in range(J):
                                eng = shift_engs[q % 4]
                                eng.dma_start(ust[16 * q:16 * q + 16, :w],
                                              u16[0:16, q:q + w])
                        if xfold == 1:
                            pv = psV.tile([16, 512], f32, name="pv")
                            nc.tensor.matmul(pv[:, :w],
                                             wts[:, WX_OFF:WX_OFF + 16],
                                             ust[:, :w], start=True, stop=True)
                            v16 = vp.tile([16, 512], cdt, name="v16")
                            nc.vector.tensor_copy(v16[:, :w], pv[:, :w])
                        elif xfold == 2:
                            # X via 8 column-offset k=16 matmuls on u16
                            pv = psV.tile([16, 512], f32, name="pv")
                            for jj in range(J):
                                nc.tensor.matmul(
                                    pv[:, :w],
                                    wts[0:16,
                                        A32_OFF + 16 * jj:A32_OFF + 16 * jj + 16],
                                    u16[0:16, 7 - jj:7 - jj + w],
                                    start=(jj == 0), stop=(jj == J - 1))
                            v16 = vp.tile([16, 512], cdt, name="v16")
                            nc.vector.tensor_copy(v16[:, :w], pv[:, :w])

                        # Pass C: Y = G Z_k + (U history)
                        for mt in range(4):
                            py = psC.tile([128, 512], f32, name="py")
                            ms0 = mt * 128
                            nc.tensor.matmul(py[:, :w], wts[:, ms0:ms0 + 128],
                                             zm[:, s + J:s + J + w],
                                             start=True, stop=False)
                            if mt >= 2:
                                nc.tensor.matmul(
                                    py[:, :w],
                                    wts[:, WG1_OFF + ms0:WG1_OFF + ms0 + 128],
                                    zm[:, KC + s + J:KC + s + J + w],
                                    start=False, stop=False)
                            if xfold:
                                nc.tensor.matmul(
                                    py[:, :w],
                                    wts[0:16, PM_OFF + ms0:PM_OFF + ms0 + 128],
                                    v16[:, :w], start=False, stop=True)
                            else:
                                nc.tensor.matmul(
                                    py[:, :w],
                                    wts[:, WU_OFF + ms0:WU_OFF + ms0 + 128],
                                    ust[:, :w], start=False, stop=True)
                            ysb = ysp.tile([128, 512], odt, name="ysb")
                            nc.vector.tensor_copy(ysb[:, :w], py[:, :w])
                            if last:
                                eng = nc.sync if mt % 2 == 0 else nc.scalar
                            else:
                                eng = nc.scalar if mt % 2 == 0 else nc.gpsimd
                            eng.dma_start(out_d[mt][:, s:s + w], ysb[:, :w])
    nc.compile()
    return nc


# ------------------------------------------------------------------ interface
def _np_dt(mode):
    if mode == "bf16":
        import ml_dtypes
        return ml_dtypes.bfloat16
    return np.float32


def _prepare(measurements, F, H, Q, R, dtype_mode):
    wts, T0 = _build_weights(F, H, Q, R)
    np_dt = _np_dt(dtype_mode)
    wtsb = np.ascontiguousarray(wts).astype(np_dt)

    meas_pad = np.zeros((TTOT, M), np.float32)
    meas_pad[:T] = measurements[:, :, 0]
    blocks = meas_pad.reshape(TTOT // L, 2, 128)      # (Ktot, ktile, 128)

    in_maps = []
    for c in range(NCORES):
        k0 = c * KB
        zc = np.zeros((2, 128, KC), np.float32)
        lo = max(0, k0 - J)
        src = blocks[lo:k0 + KB].transpose(1, 2, 0)   # (2,128,ncols)
        zc[:, :, J - (k0 - lo):] = src
        zpack = np.concatenate([zc[0], zc[1]], axis=1)  # (128, 2*KC)
        in_maps.append({"zmat": np.ascontiguousarray(zpack).astype(np_dt),
                        "wts": wtsb})
    return in_maps, T0


def _assemble(results, meas, F, H, Q, R, T0):
    chunks = []
    for c in range(NCORES):
        o = np.asarray(results[c]["out"]).astype(np.float32)  # (4,128,KB)
        Y = o.reshape(512, KB)
        chunks.append(np.ascontiguousarray(Y.T).reshape(KB * L, N))
    full = np.concatenate(chunks, axis=0)[:T]
    full[:T0] = _host_transient(meas, F, H, Q, R, T0).astype(np.float32)
    return np.ascontiguousarray(full).reshape(T, N, 1).astype(np.float32)


def run(measurements, F, H, Q, R, trace=False):
    """Returns (output, BassKernelResults)."""
    from concourse.bass_utils import run_bass_kernel_spmd

    key = (DTYPE_MODE, OUT_MODE, XFOLD)
    if _cache.get("key") != key:
        _cache["nc"] = _build_program(*key)
        _cache["key"] = key
    nc = _cache["nc"]
    in_maps, T0 = _prepare(measurements, F, H, Q, R, DTYPE_MODE)
    res = run_bass_kernel_spmd(nc, in_maps, core_ids=list(range(NCORES)),
                               trace=trace)
    out = _assemble(res.results, measurements, F, H, Q, R, T0)
    return out, res


def kernel(measurements, F, H, Q, R):
    measurements = np.asarray(measurements, dtype=np.float32)
    F = np.asarray(F, dtype=np.float32)
    H = np.asarray(H, dtype=np.float32)
    Q = np.asarray(Q, dtype=np.float32)
    R = np.asarray(R, dtype=np.float32)
    out, _ = run(measurements, F, H, Q, R, trace=False)
    return out
